# revision 1
# baseline (speedup 1.0000x reference)
"""MoE block kernel for Trainium2 (8 NeuronCores, data-parallel over batch).

Problem: B=8192, D=1024, H=256, E=16 experts, top-4 routing.
  logits = x @ route_w.T ; top4 softmax -> gates (B,E) (zeros elsewhere)
  out = sum_e gates[:,e] * relu(relu(x@W1e.T+b1e)@W2e.T+b2e)

Strategy: shard batch across 8 cores (1024 tokens each), replicate weights.
Each core computes all 16 experts densely (gates are 0 for unselected, so
dense-then-weight matches the reference exactly).

All matmuls run in fp16 (full PE rate; fp32/fp32r matmuls cannot carry sem
waits in this walrus build). The router needs near-fp32 logits so expert
selection never flips: x and route_w are split hi+lo in fp16 and the three
significant cross terms are accumulated in one PSUM group (err ~1e-7).
Expert matmuls use the fp16 hi part only (err ~1e-3, well within tolerance).

Layouts (host pre-transposed so contraction dims land on SBUF partitions):
  x_t_hi/lo (D, BL)  = fp16 split of x.T slice
  route_hi/lo (D, E) = fp16 split of route_w.T
  w1t (E, D, H), w2t (E, H, D) = fp16 weight transposes
mm1: hT[j,b] = sum_d w1t[d,j] * xT[d,b]  -> ACT relu(psum + b1[j]) -> fp16
mm2: y[b,do] = sum_j hT[j,b] * w2t[j,do] + ones[b]*b2[do] (K=1 bias matmul)
     ACT: t = relu(psum * gate[b])  (gate>=0 so relu(g*z)=g*relu(z)) -> fp16
     DVE: acc[b,do] += t   (fp16, 2x mode)
"""

import os
import sys

sys.path.insert(0, "/opt/trn_rl_repo")

import numpy as np

import concourse.bass as bass
import concourse.bacc as bacc
import concourse.mybir as mybir
import concourse.tile as tile
from concourse.bass_utils import run_bass_kernel_spmd

B, D, H, E = 8192, 1024, 256, 16
NCORES = 8
BL = B // NCORES  # 1024 tokens per core
P = 128
F32 = mybir.dt.float32
F16 = mybir.dt.float16
BBLK = 512

AX = mybir.AxisListType.X
AF = mybir.ActivationFunctionType
ALU = mybir.AluOpType


def build_nc():
    nc = bacc.Bacc("TRN2", target_bir_lowering=False, debug=False)
    x_hi = nc.declare_dram_parameter("x_hi", [D, BL], F16, isOutput=False)
    x_lo = nc.declare_dram_parameter("x_lo", [D, BL], F16, isOutput=False)
    r_hi = nc.declare_dram_parameter("r_hi", [D, E], F16, isOutput=False)
    r_lo = nc.declare_dram_parameter("r_lo", [D, E], F16, isOutput=False)
    w1t = nc.declare_dram_parameter("w1t", [E, D, H], F16, isOutput=False)
    w2t = nc.declare_dram_parameter("w2t", [E, H, D], F16, isOutput=False)
    b1 = nc.declare_dram_parameter("b1", [E, H], F32, isOutput=False)
    b2 = nc.declare_dram_parameter("b2", [E, D], F16, isOutput=False)
    out = nc.declare_dram_parameter("out", [BL, D], F16, isOutput=True)

    DT = D // P  # 8
    JT = H // P  # 2
    BT = BL // P  # 8
    NBB = BL // BBLK  # 2
    NSUB = BBLK // P  # 4
    NDO = D // BBLK  # 2

    with tile.TileContext(nc) as tc:
        with (
            tc.tile_pool(name="big", bufs=1) as big,
            tc.tile_pool(name="wts", bufs=2) as wts,
            tc.tile_pool(name="hbuf", bufs=3) as hbuf,
            tc.tile_pool(name="tbuf", bufs=6) as tbuf,
            tc.tile_pool(name="small", bufs=10) as small,
            tc.tile_pool(name="psh", bufs=2, space="PSUM") as psh_pool,
            tc.tile_pool(name="psy", bufs=4, space="PSUM") as psy_pool,
            tc.tile_pool(name="psr", bufs=2, space="PSUM") as psr_pool,
        ):
            # --- resident tensors ---
            xt_sb = big.tile([P, DT, BL], F16)  # 2MB
            nc.sync.dma_start(xt_sb, x_hi.rearrange("(o p) b -> p o b", p=P))
            xlo_sb = big.tile([P, DT, BL], F16)  # 2MB (router only)
            nc.sync.dma_start(xlo_sb, x_lo.rearrange("(o p) b -> p o b", p=P))
            rhi_sb = big.tile([P, DT, E], F16)
            nc.sync.dma_start(rhi_sb, r_hi.rearrange("(o p) e -> p o e", p=P))
            rlo_sb = big.tile([P, DT, E], F16)
            nc.sync.dma_start(rlo_sb, r_lo.rearrange("(o p) e -> p o e", p=P))
            ones_sb = big.tile([1, P], F16)
            nc.vector.memset(ones_sb, 1.0)
            acc = big.tile([P, BT, D], F16)  # 2MB output accumulator
            gates = big.tile([P, BT, E], F32)

            # --- router: logits = xhi@rhi + xhi@rlo + xlo@rhi (one group) ---
            for bt in range(BT):
                ps = psr_pool.tile([P, E], F32, tag="psr")
                groups = [(xt_sb, rhi_sb), (xt_sb, rlo_sb), (xlo_sb, rhi_sb)]
                n_mm = len(groups) * DT
                k = 0
                for xs, rs in groups:
                    for dt_i in range(DT):
                        nc.tensor.matmul(
                            ps,
                            lhsT=xs[:, dt_i, bt * P : (bt + 1) * P],
                            rhs=rs[:, dt_i, :],
                            start=(k == 0),
                            stop=(k == n_mm - 1),
                        )
                        k += 1
                logits = small.tile([P, E], F32, tag="logits")
                nc.vector.tensor_copy(logits, ps)
                m1 = small.tile([P, 1], F32, tag="m1")
                nc.vector.reduce_max(m1, logits, axis=AX)
                neg_m1 = small.tile([P, 1], F32, tag="negm1")
                nc.vector.tensor_scalar_mul(neg_m1, m1, -1.0)
                # knock out top-3, leaving mcur = 4th-largest logit
                tmp = small.tile([P, E], F32, tag="tmp")
                nc.vector.tensor_copy(tmp, logits)
                mcur = m1
                for it in range(3):
                    mask = small.tile([P, E], F32, tag=f"mask{it}")
                    nc.vector.tensor_scalar(mask, tmp, mcur, None, op0=ALU.is_ge)
                    nc.vector.scalar_tensor_tensor(
                        tmp, mask, -1e30, tmp, op0=ALU.mult, op1=ALU.add
                    )
                    mnext = small.tile([P, 1], F32, tag=f"mnext{it}")
                    nc.vector.reduce_max(mnext, tmp, axis=AX)
                    mcur = mnext
                maskt = small.tile([P, E], F32, tag="maskt")
                nc.vector.tensor_scalar(maskt, logits, mcur, None, op0=ALU.is_ge)
                expv = small.tile([P, E], F32, tag="expv")
                nc.scalar.activation(expv, logits, AF.Exp, bias=neg_m1, scale=1.0)
                expm = small.tile([P, E], F32, tag="expm")
                nc.vector.tensor_mul(expm, expv, maskt)
                ssum = small.tile([P, 1], F32, tag="ssum")
                nc.vector.reduce_sum(ssum, expm, axis=AX)
                rinv = small.tile([P, 1], F32, tag="rinv")
                nc.vector.reciprocal(rinv, ssum)
                nc.vector.tensor_scalar_mul(gates[:, bt, :], expm, rinv)

            # --- expert loop ---
            for e in range(E):
                w1_sb = wts.tile([P, DT, H], F16, tag="w1")
                nc.sync.dma_start(w1_sb, w1t[e].rearrange("(o p) h -> p o h", p=P))
                w2_sb = wts.tile([P, JT, D], F16, tag="w2")
                nc.sync.dma_start(w2_sb, w2t[e].rearrange("(o p) d -> p o d", p=P))
                b1_sb = wts.tile([P, JT], F32, tag="b1")
                nc.sync.dma_start(b1_sb, b1[e].rearrange("(o p) -> p o", p=P))
                b2row = wts.tile([1, D], F16, tag="b2")
                nc.sync.dma_start(b2row, b2[e][None, :])

                for bb in range(NBB):
                    hT = hbuf.tile([P, JT, BBLK], F16, tag="hT")
                    for jt in range(JT):
                        psh = psh_pool.tile([P, BBLK], F32, tag="psh")
                        for dt_i in range(DT):
                            nc.tensor.matmul(
                                psh,
                                lhsT=w1_sb[:, dt_i, jt * P : (jt + 1) * P],
                                rhs=xt_sb[:, dt_i, bb * BBLK : (bb + 1) * BBLK],
                                start=(dt_i == 0),
                                stop=(dt_i == DT - 1),
                            )
                        nc.scalar.activation(
                            hT[:, jt, :], psh, AF.Relu, bias=b1_sb[:, jt : jt + 1]
                        )
                    for bsub in range(NSUB):
                        bt = bb * NSUB + bsub
                        for dot in range(NDO):
                            psy = psy_pool.tile([P, BBLK], F32, tag="psy")
                            for jt in range(JT):
                                nc.tensor.matmul(
                                    psy,
                                    lhsT=hT[:, jt, bsub * P : (bsub + 1) * P],
                                    rhs=w2_sb[:, jt, dot * BBLK : (dot + 1) * BBLK],
                                    start=(jt == 0),
                                    stop=False,
                                )
                            nc.tensor.matmul(
                                psy,
                                lhsT=ones_sb,
                                rhs=b2row[:, dot * BBLK : (dot + 1) * BBLK],
                                start=False,
                                stop=True,
                            )
                            t = tbuf.tile([P, BBLK], F16, tag="t")
                            nc.scalar.activation(
                                t, psy, AF.Relu, scale=gates[:, bt, e : e + 1]
                            )
                            oslice = acc[:, bt, dot * BBLK : (dot + 1) * BBLK]
                            if e == 0:
                                nc.vector.tensor_copy(oslice, t)
                            else:
                                nc.vector.tensor_add(oslice, oslice, t)

            nc.sync.dma_start(out.rearrange("(o p) d -> p o d", p=P), acc)
    nc.compile()
    return nc


_NC_CACHE = None


def _get_nc():
    global _NC_CACHE
    if _NC_CACHE is None:
        _NC_CACHE = build_nc()
    return _NC_CACHE


def _split16(a):
    hi = a.astype(np.float16)
    lo = (a - hi.astype(np.float32)).astype(np.float16)
    return np.ascontiguousarray(hi), np.ascontiguousarray(lo)


def _prep_in_maps(x, route_w, w1, b1, w2, b2):
    x_t = np.asarray(x, dtype=np.float32).T  # (D, B)
    x_hi, x_lo = _split16(x_t)
    r_hi, r_lo = _split16(np.asarray(route_w, dtype=np.float32).T)
    w1t = np.ascontiguousarray(
        np.asarray(w1, dtype=np.float32).transpose(0, 2, 1).astype(np.float16)
    )
    w2t = np.ascontiguousarray(
        np.asarray(w2, dtype=np.float32).transpose(0, 2, 1).astype(np.float16)
    )
    b1 = np.ascontiguousarray(np.asarray(b1, dtype=np.float32))
    b2 = np.ascontiguousarray(np.asarray(b2, dtype=np.float32).astype(np.float16))
    in_maps = []
    for c in range(NCORES):
        sl = slice(c * BL, (c + 1) * BL)
        in_maps.append(
            {
                "x_hi": np.ascontiguousarray(x_hi[:, sl]),
                "x_lo": np.ascontiguousarray(x_lo[:, sl]),
                "r_hi": r_hi,
                "r_lo": r_lo,
                "w1t": w1t,
                "w2t": w2t,
                "b1": b1,
                "b2": b2,
            }
        )
    return in_maps


def run(x, route_w, w1, b1, w2, b2, trace=False, **trace_kw):
    nc = _get_nc()
    in_maps = _prep_in_maps(x, route_w, w1, b1, w2, b2)
    res = run_bass_kernel_spmd(
        nc, in_maps, list(range(NCORES)), trace=trace, **trace_kw
    )
    out = np.concatenate(
        [r["out"].astype(np.float32) for r in res.results], axis=0
    )
    return out, res


def kernel(x, route_w, w1, b1, w2, b2):
    out, _ = run(x, route_w, w1, b1, w2, b2, trace=False)
    return out



# revision 12
# speedup vs baseline: 1.4502x; 1.4502x over previous
"""Sparse MoE block kernel for Trainium2 (8 NeuronCores, data-parallel batch).

Problem: B=8192, D=1024, H=256, E=16 experts, top-4 routing.
  logits = x @ route_w.T ; top4 softmax -> gates ; out = sum_e gate_e * FFN_e(x)

Strategy (vs dense baseline): exploit top-4 sparsity. Each core owns 1024
tokens. The host computes only the dispatch SCHEDULE (which token goes to
which expert, exactly matching the reference's top_k picks); all values are
computed on device:
  - router: logits (fp16 hi+lo route operand), top-4 mask, softmax -> gates
    (token-major, fp32), written to a small HBM table [1024, 64].
  - dispatch: dma_gather (transpose mode) packs the selected tokens'
    x-columns per expert into a slot-major xT layout [128, 8, S].
  - per-expert gates: dma_gather of each expert's token gate rows; the
    ACT scale reads column e directly.
  - FFN per expert on exactly its selected tokens (padded to the max count
    across the 8 cores, multiple of 16): mm1 (hT, H-major) -> relu+b1 ->
    mm2 (token-major psum, b2 via rank-1 ones matmul) -> relu * gate.
  - combine: dma_scatter_add adds each expert's gated rows into the HBM
    output at the token row (fp16 CCE add). Padding slots scatter into a
    dummy row 1024 which the host drops. Scatters are per expert, so
    destination indices within one scatter instruction are unique (no
    concurrent read-modify-write races on a row).

Numerical notes: slot gates are UNMASKED softmax terms exp(l-m)/Z where only
Z uses the device's own top-4 mask. If the device's 4th pick disagrees with
the host schedule (logit gap below fp16 noise), Z changes by O(gap) only, so
the result stays within tolerance. Expert matmuls run fp16 (err ~1e-3).
"""

import os
import sys

sys.path.insert(0, "/opt/trn_rl_repo")

import numpy as np

import concourse.bass as bass
import concourse.bacc as bacc
import concourse.mybir as mybir
import concourse.tile as tile
from concourse.bass_utils import run_bass_kernel_spmd

B, D, H, E, K = 8192, 1024, 256, 16, 4
NCORES = 8
BL = B // NCORES  # 1024 tokens per core
P = 128
F32 = mybir.dt.float32
F16 = mybir.dt.float16
I16 = mybir.dt.int16
DUMMY = BL  # scatter row for padding slots

AX = mybir.AxisListType.X
AF = mybir.ActivationFunctionType
ALU = mybir.AluOpType


def _cdiv(a, b):
    return (a + b - 1) // b


def build_nc(C):
    """C: tuple of 16 per-expert slot capacities (each %16==0, sum %128==0)."""
    S = sum(C)
    offs = np.concatenate([[0], np.cumsum(C)]).astype(int)
    PIECE = 512  # transpose dma_gather crashes HW above ~768 idxs
    pieces = [(p0, min(PIECE, S - p0)) for p0 in range(0, S, PIECE)]
    nch = [_cdiv(c, P) for c in C]
    NCH = max(nch)
    DT = D // P  # 8
    JT = H // P  # 2

    nc = bacc.Bacc("TRN2", target_bir_lowering=False, debug=False)
    x_row = nc.declare_dram_parameter("x_row", [BL, D], F16, isOutput=False)
    x_t = nc.declare_dram_parameter("x_t", [D, BL], F16, isOutput=False)
    r_cat = nc.declare_dram_parameter("r_cat", [D, 2 * E], F16, isOutput=False)
    w1t = nc.declare_dram_parameter("w1t", [E, D, H], F16, isOutput=False)
    w2t = nc.declare_dram_parameter("w2t", [E, H, D], F16, isOutput=False)
    b1 = nc.declare_dram_parameter("b1", [E, H], F32, isOutput=False)
    b2 = nc.declare_dram_parameter("b2", [E, D], F16, isOutput=False)
    gidx = nc.declare_dram_parameter("gidx", [128, S // 16], I16, isOutput=False)
    sidx = nc.declare_dram_parameter("sidx", [128, S // 16], I16, isOutput=False)
    out = nc.declare_dram_parameter("out", [BL + 1, D], F16, isOutput=True)
    gates_hbm = nc.dram_tensor([BL, 64], F32, kind="Internal")

    with tile.TileContext(nc) as tc:
        with (
            tc.tile_pool(name="big", bufs=1) as big,
            tc.tile_pool(name="xg", bufs=3) as xgpool,
            tc.tile_pool(name="wts", bufs=3) as wts,
            tc.tile_pool(name="gg", bufs=3) as ggpool,
            tc.tile_pool(name="yb", bufs=3) as ypool,
            tc.tile_pool(name="small", bufs=10) as small,
            tc.tile_pool(name="psr", bufs=1, space="PSUM") as psr_pool,
            tc.tile_pool(name="psh", bufs=2, space="PSUM") as psh_pool,
            tc.tile_pool(name="psy", bufs=4, space="PSUM") as psy_pool,
        ):
            # --- resident tensors ---
            xt_sb = big.tile([P, DT, BL], F16)  # 2MB, router lhsT
            nc.sync.dma_start(xt_sb, x_t.rearrange("(o p) t -> p o t", p=P))
            rcat_sb = big.tile([P, DT, 2 * E], F16)
            nc.sync.dma_start(rcat_sb, r_cat.rearrange("(o p) e -> p o e", p=P))
            gidx_sb = big.tile([128, S // 16], I16)
            nc.sync.dma_start(gidx_sb, gidx[:, :])
            sidx_sb = big.tile([128, S // 16], I16)
            nc.sync.dma_start(sidx_sb, sidx[:, :])
            ones_sb = big.tile([1, P], F16)
            nc.vector.memset(ones_sb, 1.0)
            zrow = big.tile([P, D], F16)
            nc.vector.memset(zrow, 0.0)
            gates_sb = big.tile([P, BL // P, 64], F32)
            nc.vector.memset(gates_sb, 0.0)
            hT = big.tile([P, JT, S], F16)  # global packed h^T

            # --- zero the output (scatter-add target) ---
            for i in range(BL // P):
                nc.sync.dma_start(out[i * P : (i + 1) * P, :], zrow)
            nc.sync.dma_start(out[BL : BL + 1, :], zrow[0:1, :])

            # --- router (token-major): logits = x@(r_hi|r_lo), top4 softmax
            for bt in range(BL // P):
                ps = psr_pool.tile([P, E], F32, tag="psr")
                k = 0
                for half in range(2):
                    for dt_i in range(DT):
                        nc.tensor.matmul(
                            ps,
                            lhsT=xt_sb[:, dt_i, bt * P : (bt + 1) * P],
                            rhs=rcat_sb[:, dt_i, half * E : (half + 1) * E],
                            start=(k == 0),
                            stop=(k == 2 * DT - 1),
                        )
                        k += 1
                logits = small.tile([P, E], F32, tag="logits")
                nc.vector.tensor_copy(logits, ps)
                m1 = small.tile([P, 1], F32, tag="m1")
                nc.vector.reduce_max(m1, logits, axis=AX)
                neg_m1 = small.tile([P, 1], F32, tag="negm1")
                nc.vector.tensor_scalar_mul(neg_m1, m1, -1.0)
                # knock out top-3, leaving mcur = 4th-largest logit
                tmp = small.tile([P, E], F32, tag="tmp")
                nc.vector.tensor_copy(tmp, logits)
                mcur = m1
                for it in range(K - 1):
                    mask = small.tile([P, E], F32, tag=f"mask{it}")
                    nc.vector.tensor_scalar(mask, tmp, mcur, None, op0=ALU.is_ge)
                    nc.vector.scalar_tensor_tensor(
                        tmp, mask, -1e30, tmp, op0=ALU.mult, op1=ALU.add
                    )
                    mnext = small.tile([P, 1], F32, tag=f"mnext{it}")
                    nc.vector.reduce_max(mnext, tmp, axis=AX)
                    mcur = mnext
                maskt = small.tile([P, E], F32, tag="maskt")
                nc.vector.tensor_scalar(maskt, logits, mcur, None, op0=ALU.is_ge)
                expv = small.tile([P, E], F32, tag="expv")
                nc.scalar.activation(expv, logits, AF.Exp, bias=neg_m1, scale=1.0)
                expm = small.tile([P, E], F32, tag="expm")
                nc.vector.tensor_mul(expm, expv, maskt)
                ssum = small.tile([P, 1], F32, tag="ssum")
                nc.vector.reduce_sum(ssum, expm, axis=AX)
                rinv = small.tile([P, 1], F32, tag="rinv")
                nc.vector.reciprocal(rinv, ssum)
                # UNMASKED slot gates: exp(l - m)/Z
                nc.vector.tensor_scalar_mul(gates_sb[:, bt, 0:E], expv, rinv)
            nc.sync.dma_start(
                gates_hbm.rearrange("(o p) f -> p o f", p=P), gates_sb
            )

            # --- slot dispatch: gather x columns per 1024-slot piece ---
            xg_tiles = {}

            def emit_xg(p):
                p0, plen = pieces[p]
                tag = "xg" if plen == PIECE else "xgtail"
                t = xgpool.tile([P, DT, plen], F16, tag=tag)
                nc.gpsimd.dma_gather(
                    t[:, :, :],
                    x_row[:, :],
                    gidx_sb[:, p0 // 16 : (p0 + plen) // 16],
                    plen,
                    plen,
                    D,
                    transpose=True,
                )
                xg_tiles[p] = t

            emit_xg(0)
            if len(pieces) > 1:
                emit_xg(1)
            emitted = min(1, len(pieces) - 1)

            def mm1_chunks(lo, hi):
                """Split [lo,hi) at the gather-piece grid, then into <=512."""
                res = []
                a = lo
                while a < hi:
                    b = min(hi, (a // PIECE + 1) * PIECE)
                    while a < b:
                        c = min(b, a + 512)
                        res.append((a, c))
                        a = c
                return res

            for e in range(E):
                lo, hi = int(offs[e]), int(offs[e + 1])
                last_piece = (hi - 1) // PIECE
                while emitted < last_piece:
                    emitted += 1
                    emit_xg(emitted)

                w1_sb = wts.tile([P, DT, H], F16, tag="w1")
                nc.sync.dma_start(w1_sb, w1t[e].rearrange("(o p) h -> p o h", p=P))
                w2_sb = wts.tile([P, JT, D], F16, tag="w2")
                nc.sync.dma_start(w2_sb, w2t[e].rearrange("(o p) d -> p o d", p=P))
                b1_sb = wts.tile([P, JT], F32, tag="b1")
                nc.sync.dma_start(b1_sb, b1[e].rearrange("(o p) -> p o", p=P))
                b2_sb = wts.tile([1, D], F16, tag="b2")
                nc.sync.dma_start(b2_sb, b2[e][None, :])

                gg_e = ggpool.tile([P, nch[e], 64], F32, tag=f"gg{nch[e]}")
                nc.gpsimd.dma_gather(
                    gg_e[:, :, :],
                    gates_hbm[:, :],
                    gidx_sb[:, lo // 16 : hi // 16],
                    C[e],
                    C[e],
                    64,
                    transpose=False,
                )

                # mm1: hT[:, jt, lo:hi] = relu(w1^T-contract x_slots + b1)
                for jt in range(JT):
                    for a, bnd in mm1_chunks(lo, hi):
                        ln = bnd - a
                        p = a // PIECE
                        psh = psh_pool.tile([P, 512], F32, tag="psh")
                        for dt_i in range(DT):
                            nc.tensor.matmul(
                                psh[:, 0:ln],
                                lhsT=w1_sb[:, dt_i, jt * P : (jt + 1) * P],
                                rhs=xg_tiles[p][:, dt_i, a - PIECE * p : bnd - PIECE * p],
                                start=(dt_i == 0),
                                stop=(dt_i == DT - 1),
                            )
                        nc.scalar.activation(
                            hT[:, jt, a:bnd],
                            psh[:, 0:ln],
                            AF.Relu,
                            bias=b1_sb[:, jt : jt + 1],
                        )

                # mm2: y = relu(gate * (hT^T w2 + b2)) ; token(slot)-major psum
                y_e = ypool.tile([P, NCH, D], F16, tag="y")
                tail = C[e] - P * (nch[e] - 1)
                if tail < P:  # ragged last chunk: init rows the ACT won't write
                    nc.vector.memset(y_e[:, nch[e] - 1, :], 0.0)
                if nch[e] < NCH:
                    nc.vector.memset(y_e[:, nch[e] : NCH, :], 0.0)
                for i in range(nch[e]):
                    la = P * i
                    lb = min(la + P, C[e])
                    ln = lb - la
                    for dc in range(2):
                        psy = psy_pool.tile([P, 512], F32, tag="psy")
                        for jt in range(JT):
                            nc.tensor.matmul(
                                psy[0:ln, :],
                                lhsT=hT[:, jt, lo + la : lo + lb],
                                rhs=w2_sb[:, jt, dc * 512 : (dc + 1) * 512],
                                start=(jt == 0),
                                stop=False,
                            )
                        nc.tensor.matmul(
                            psy[0:ln, :],
                            lhsT=ones_sb[0:1, 0:ln],
                            rhs=b2_sb[0:1, dc * 512 : (dc + 1) * 512],
                            start=False,
                            stop=True,
                        )
                        nc.scalar.activation(
                            y_e[0:ln, i, dc * 512 : (dc + 1) * 512],
                            psy[0:ln, :],
                            AF.Relu,
                            scale=gg_e[0:ln, i, e : e + 1],
                        )

                nc.gpsimd.dma_scatter_add(
                    out[:, :],
                    y_e[:, 0 : nch[e], :],
                    sidx_sb[:, lo // 16 : hi // 16],
                    C[e],
                    C[e],
                    D,
                )
    nc.compile()
    return nc


_NC_CACHE = {}


def _get_nc(C):
    key = tuple(C)
    if key not in _NC_CACHE:
        _NC_CACHE[key] = build_nc(key)
    return _NC_CACHE[key]


def _topk_idx(x, route_w):
    """Top-4 expert ids per token, matching the reference's jax top_k."""
    try:
        import jax

        cpu = jax.devices("cpu")[0]
        with jax.default_device(cpu):
            f = jax.jit(
                lambda x, r: jax.lax.top_k(x @ r.T, K)[1], backend="cpu"
            )
            return np.asarray(f(x, route_w))
    except Exception:
        l = x.astype(np.float32) @ route_w.astype(np.float32).T
        return np.argsort(-l, axis=1, kind="stable")[:, :K].astype(np.int32)


def _schedule(x, route_w):
    """Build per-core dispatch schedule. Returns (C, per-core arrays)."""
    idx = _topk_idx(np.asarray(x, np.float32), np.asarray(route_w, np.float32))
    sel = np.zeros((NCORES, BL, E), dtype=bool)
    rows = np.arange(BL)
    for c in range(NCORES):
        sel[c, rows[:, None].repeat(K, 1), idx[c * BL : (c + 1) * BL]] = True
    counts = sel.sum(axis=1)  # (NCORES, E)
    C = ((counts.max(axis=0) + 15) // 16 * 16).astype(int)
    C = np.maximum(C, 16)
    deficit = (-C.sum()) % 128
    C[E - 1] += deficit  # multiple of 16 since all terms are
    S = int(C.sum())
    offs = np.concatenate([[0], np.cumsum(C)]).astype(int)

    per_core = []
    for c in range(NCORES):
        g = np.zeros(S, dtype=np.int16)
        s = np.full(S, DUMMY, dtype=np.int16)
        for e in range(E):
            toks = np.nonzero(sel[c, :, e])[0]
            n = len(toks)
            g[offs[e] : offs[e] + n] = toks
            s[offs[e] : offs[e] + n] = toks
        # idx j -> [j%16, j//16], replicated across the 8 gpsimd cores
        wrap = lambda a: np.ascontiguousarray(
            np.tile(a.reshape(S // 16, 16).T, (8, 1))
        )
        per_core.append((wrap(g), wrap(s)))
    return tuple(int(v) for v in C), per_core


def _prep_in_maps(x, route_w, w1, b1, w2, b2, C, per_core):
    x = np.asarray(x, dtype=np.float32)
    rw = np.asarray(route_w, dtype=np.float32)
    r_hi = rw.T.astype(np.float16)
    r_lo = (rw.T - r_hi.astype(np.float32)).astype(np.float16)
    r_cat = np.ascontiguousarray(np.concatenate([r_hi, r_lo], axis=1))
    w1t = np.ascontiguousarray(
        np.asarray(w1, np.float32).transpose(0, 2, 1).astype(np.float16)
    )
    w2t = np.ascontiguousarray(
        np.asarray(w2, np.float32).transpose(0, 2, 1).astype(np.float16)
    )
    b1 = np.ascontiguousarray(np.asarray(b1, np.float32))
    b2 = np.ascontiguousarray(np.asarray(b2, np.float32).astype(np.float16))
    in_maps = []
    for c in range(NCORES):
        xc = x[c * BL : (c + 1) * BL]
        gw, sw = per_core[c]
        in_maps.append(
            {
                "x_row": np.ascontiguousarray(xc.astype(np.float16)),
                "x_t": np.ascontiguousarray(xc.T.astype(np.float16)),
                "r_cat": r_cat,
                "w1t": w1t,
                "w2t": w2t,
                "b1": b1,
                "b2": b2,
                "gidx": gw,
                "sidx": sw,
            }
        )
    return in_maps


def run(x, route_w, w1, b1, w2, b2, trace=False, **trace_kw):
    C, per_core = _schedule(x, route_w)
    nc = _get_nc(C)
    in_maps = _prep_in_maps(x, route_w, w1, b1, w2, b2, C, per_core)
    res = run_bass_kernel_spmd(
        nc, in_maps, list(range(NCORES)), trace=trace, **trace_kw
    )
    out = np.concatenate(
        [r["out"][:BL].astype(np.float32) for r in res.results], axis=0
    )
    return out, res


def kernel(x, route_w, w1, b1, w2, b2):
    out, _ = run(x, route_w, w1, b1, w2, b2, trace=False)
    return out


# revision 15
# speedup vs baseline: 1.4512x; 1.0007x over previous
"""Sparse MoE block kernel for Trainium2 (8 NeuronCores, data-parallel batch).

Problem: B=8192, D=1024, H=256, E=16 experts, top-4 routing.
  logits = x @ route_w.T ; top4 softmax -> gates ; out = sum_e gate_e * FFN_e(x)

Strategy (vs dense baseline): exploit top-4 sparsity. Each core owns 1024
tokens. The host computes only the dispatch SCHEDULE (which token goes to
which expert, exactly matching the reference's top_k picks); all values are
computed on device:
  - router: logits (fp16 hi+lo route operand), top-4 mask, softmax -> gates
    (token-major, fp32), written to a small HBM table [1024, 64].
  - dispatch: dma_gather (transpose mode) packs the selected tokens'
    x-columns per expert into a slot-major xT layout [128, 8, S].
  - per-expert gates: dma_gather of each expert's token gate rows; the
    ACT scale reads column e directly.
  - FFN per expert on exactly its selected tokens (padded to the max count
    across the 8 cores, multiple of 16): mm1 (hT, H-major) -> relu+b1 ->
    mm2 (token-major psum, b2 via rank-1 ones matmul) -> relu * gate.
  - combine: dma_scatter_add adds each expert's gated rows into the HBM
    output at the token row (fp16 CCE add). Padding slots scatter into a
    dummy row 1024 which the host drops. Scatters are per expert, so
    destination indices within one scatter instruction are unique (no
    concurrent read-modify-write races on a row).

Numerical notes: slot gates are UNMASKED softmax terms exp(l-m)/Z where only
Z uses the device's own top-4 mask. If the device's 4th pick disagrees with
the host schedule (logit gap below fp16 noise), Z changes by O(gap) only, so
the result stays within tolerance. Expert matmuls run fp16 (err ~1e-3).
"""

import os
import sys

sys.path.insert(0, "/opt/trn_rl_repo")

import numpy as np

import concourse.bass as bass
import concourse.bacc as bacc
import concourse.mybir as mybir
import concourse.tile as tile
from concourse.bass_utils import run_bass_kernel_spmd

B, D, H, E, K = 8192, 1024, 256, 16, 4
NCORES = 8
BL = B // NCORES  # 1024 tokens per core
P = 128
F32 = mybir.dt.float32
F16 = mybir.dt.float16
I16 = mybir.dt.int16
DUMMY = BL  # scatter row for padding slots

AX = mybir.AxisListType.X
AF = mybir.ActivationFunctionType
ALU = mybir.AluOpType


def _cdiv(a, b):
    return (a + b - 1) // b


def _layout(C):
    """Slot-space layout derived from capacities.

    Pieces pack whole experts up to MAXP idxs (transpose dma_gather crashes
    above ~768), padded to %128. gg groups pack 128-aligned per-expert gate
    segments up to 1024 idxs per gather instruction.
    """
    MAXP = 640
    nch = [_cdiv(c, P) for c in C]
    pieces = []  # (p0, size)
    piece_of = {}  # e -> piece index
    offs = {}  # e -> global slot offset
    cur_es, cur_n = [], 0
    elist = list(range(E))
    p0 = 0

    def flush():
        nonlocal p0, cur_es, cur_n
        if not cur_es:
            return
        size = _cdiv(cur_n, P) * P
        for e_, o_ in cur_es:
            piece_of[e_] = len(pieces)
            offs[e_] = p0 + o_
        pieces.append((p0, size))
        p0 += size
        cur_es, cur_n = [], 0

    for e in elist:
        if cur_n + C[e] > MAXP:
            flush()
        cur_es.append((e, cur_n))
        cur_n += C[e]
    flush()
    S = p0
    # gg groups: experts packed so sum of 128*nch <= 1024
    groups = []  # list of (row0_global, [(e, local_row_off)])
    cur, rows, row0 = [], 0, 0
    for e in elist:
        if (rows + nch[e]) * P > 1024 and cur:
            groups.append((row0, cur))
            row0 += rows
            cur, rows = [], 0
        cur.append((e, rows))
        rows += nch[e]
    if cur:
        groups.append((row0, cur))
    G = P * sum(nch)
    return pieces, piece_of, offs, S, nch, groups, G


def build_nc(C):
    """C: tuple of 16 per-expert slot capacities (each %16==0)."""
    pieces, piece_of, offs, S, nch, groups, G = _layout(C)
    NCH = max(nch)
    DT = D // P  # 8
    JT = H // P  # 2

    nc = bacc.Bacc("TRN2", target_bir_lowering=False, debug=False)
    x_row = nc.declare_dram_parameter("x_row", [BL, D], F16, isOutput=False)
    x_t = nc.declare_dram_parameter("x_t", [D, BL], F16, isOutput=False)
    r_cat = nc.declare_dram_parameter("r_cat", [D, 2 * E], F16, isOutput=False)
    w1t = nc.declare_dram_parameter("w1t", [E, D, H], F16, isOutput=False)
    w2t = nc.declare_dram_parameter("w2t", [E, H, D], F16, isOutput=False)
    b1 = nc.declare_dram_parameter("b1", [E, H], F32, isOutput=False)
    b2 = nc.declare_dram_parameter("b2", [E, P, D], F16, isOutput=False)
    gidx = nc.declare_dram_parameter("gidx", [128, S // 16], I16, isOutput=False)
    sidx = nc.declare_dram_parameter("sidx", [128, S // 16], I16, isOutput=False)
    ggidx = nc.declare_dram_parameter("ggidx", [128, G // 16], I16, isOutput=False)
    out = nc.declare_dram_parameter("out", [BL + 1, D], F16, isOutput=True)
    gates_hbm = nc.dram_tensor([BL, 64], F32, kind="Internal")

    with tile.TileContext(nc) as tc:
        with (
            tc.tile_pool(name="big", bufs=1) as big,
            tc.tile_pool(name="xg", bufs=3) as xgpool,
            tc.tile_pool(name="wts", bufs=3) as wts,
            tc.tile_pool(name="gg", bufs=3) as ggpool,
            tc.tile_pool(name="yb", bufs=3) as ypool,
            tc.tile_pool(name="small", bufs=10) as small,
            tc.tile_pool(name="psr", bufs=1, space="PSUM") as psr_pool,
            tc.tile_pool(name="psh", bufs=2, space="PSUM") as psh_pool,
            tc.tile_pool(name="psy", bufs=4, space="PSUM") as psy_pool,
        ):
            # --- resident tensors ---
            xt_sb = big.tile([P, DT, BL], F16)  # 2MB, router lhsT
            nc.sync.dma_start(xt_sb, x_t.rearrange("(o p) t -> p o t", p=P))
            rcat_sb = big.tile([P, DT, 2 * E], F16)
            nc.sync.dma_start(rcat_sb, r_cat.rearrange("(o p) e -> p o e", p=P))
            gidx_sb = big.tile([128, S // 16], I16)
            nc.sync.dma_start(gidx_sb, gidx[:, :])
            sidx_sb = big.tile([128, S // 16], I16)
            nc.sync.dma_start(sidx_sb, sidx[:, :])
            ggidx_sb = big.tile([128, G // 16], I16)
            nc.sync.dma_start(ggidx_sb, ggidx[:, :])
            ones_sb = big.tile([1, P], F16)
            nc.vector.memset(ones_sb, 1.0)
            zrow = big.tile([P, D], F16)
            nc.vector.memset(zrow, 0.0)
            gates_sb = big.tile([P, BL // P, 64], F32)
            nc.vector.memset(gates_sb, 0.0)
            hT = big.tile([P, JT, S], F16)  # global packed h^T

            # --- zero the output (scatter-add target) ---
            for i in range(BL // P):
                nc.sync.dma_start(out[i * P : (i + 1) * P, :], zrow)
            nc.sync.dma_start(out[BL : BL + 1, :], zrow[0:1, :])

            # --- router (token-major): logits = x@(r_hi|r_lo), top4 softmax
            for bt in range(BL // P):
                ps = psr_pool.tile([P, 2 * E], F32, tag="psr")
                for dt_i in range(DT):
                    nc.tensor.matmul(
                        ps,
                        lhsT=xt_sb[:, dt_i, bt * P : (bt + 1) * P],
                        rhs=rcat_sb[:, dt_i, :],
                        start=(dt_i == 0),
                        stop=(dt_i == DT - 1),
                    )
                l2 = small.tile([P, 2 * E], F32, tag="l2")
                nc.vector.tensor_copy(l2, ps)
                logits = small.tile([P, E], F32, tag="logits")
                nc.vector.tensor_add(logits, l2[:, 0:E], l2[:, E : 2 * E])
                m1 = small.tile([P, 1], F32, tag="m1")
                nc.vector.reduce_max(m1, logits, axis=AX)
                neg_m1 = small.tile([P, 1], F32, tag="negm1")
                nc.vector.tensor_scalar_mul(neg_m1, m1, -1.0)
                # knock out top-3, leaving mcur = 4th-largest logit
                tmp = small.tile([P, E], F32, tag="tmp")
                nc.vector.tensor_copy(tmp, logits)
                mcur = m1
                for it in range(K - 1):
                    mask = small.tile([P, E], F32, tag=f"mask{it}")
                    nc.vector.tensor_scalar(mask, tmp, mcur, None, op0=ALU.is_ge)
                    nc.vector.scalar_tensor_tensor(
                        tmp, mask, -1e30, tmp, op0=ALU.mult, op1=ALU.add
                    )
                    mnext = small.tile([P, 1], F32, tag=f"mnext{it}")
                    nc.vector.reduce_max(mnext, tmp, axis=AX)
                    mcur = mnext
                maskt = small.tile([P, E], F32, tag="maskt")
                nc.vector.tensor_scalar(maskt, logits, mcur, None, op0=ALU.is_ge)
                expv = small.tile([P, E], F32, tag="expv")
                nc.scalar.activation(expv, logits, AF.Exp, bias=neg_m1, scale=1.0)
                expm = small.tile([P, E], F32, tag="expm")
                nc.vector.tensor_mul(expm, expv, maskt)
                ssum = small.tile([P, 1], F32, tag="ssum")
                nc.vector.reduce_sum(ssum, expm, axis=AX)
                rinv = small.tile([P, 1], F32, tag="rinv")
                nc.vector.reciprocal(rinv, ssum)
                # UNMASKED slot gates: exp(l - m)/Z
                nc.vector.tensor_scalar_mul(gates_sb[:, bt, 0:E], expv, rinv)
            nc.sync.dma_start(
                gates_hbm.rearrange("(o p) f -> p o f", p=P), gates_sb
            )

            # --- slot dispatch: gather x columns per 1024-slot piece ---
            xg_tiles = {}

            def emit_xg(p):
                p0, plen = pieces[p]
                t = xgpool.tile([P, DT, plen], F16, tag=f"xg{plen}")
                nc.gpsimd.dma_gather(
                    t[:, :, :],
                    x_row[:, :],
                    gidx_sb[:, p0 // 16 : (p0 + plen) // 16],
                    plen,
                    plen,
                    D,
                    transpose=True,
                )
                xg_tiles[p] = t

            emit_xg(0)
            if len(pieces) > 1:
                emit_xg(1)
            emitted = min(1, len(pieces) - 1)

            piece_end = {i: p0 + sz for i, (p0, sz) in enumerate(pieces)}

            def piece_at(a):
                for i, (p0, sz) in enumerate(pieces):
                    if p0 <= a < p0 + sz:
                        return i
                raise AssertionError(a)

            def mm1_chunks(lo, hi):
                """Split [lo,hi) at gather-piece boundaries, then into <=512."""
                res = []
                a = lo
                while a < hi:
                    pi = piece_at(a)
                    b = min(hi, piece_end[pi])
                    while a < b:
                        c = min(b, a + 512)
                        res.append((a, c, pi))
                        a = c
                return res

            # gate gathers: one instruction per <=1024-idx group, segments
            # 128-aligned per expert so ACT scale slices stay partition-aligned
            gg_of = {}
            for row0, members in groups:
                rows = sum(nch[e] for e, _ in members)
                gg_t = ggpool.tile([P, rows, 64], F32, tag=f"gg{rows}")
                nc.gpsimd.dma_gather(
                    gg_t[:, :, :],
                    gates_hbm[:, :],
                    ggidx_sb[:, P * row0 // 16 : P * (row0 + rows) // 16],
                    P * rows,
                    P * rows,
                    64,
                    transpose=False,
                )
                for e, loc in members:
                    gg_of[e] = (gg_t, loc)

            for e in range(E):
                lo, hi = int(offs[e]), int(offs[e]) + C[e]
                last_piece = piece_of[e]
                while emitted < last_piece:
                    emitted += 1
                    emit_xg(emitted)

                w1_sb = wts.tile([P, DT, H], F16, tag="w1")
                nc.sync.dma_start(w1_sb, w1t[e].rearrange("(o p) h -> p o h", p=P))
                w2_sb = wts.tile([P, JT, D], F16, tag="w2")
                nc.sync.dma_start(w2_sb, w2t[e].rearrange("(o p) d -> p o d", p=P))
                b1_sb = wts.tile([P, JT], F32, tag="b1")
                nc.sync.dma_start(b1_sb, b1[e].rearrange("(o p) -> p o", p=P))
                b2_sb = wts.tile([P, D], F16, tag="b2")
                nc.sync.dma_start(b2_sb, b2[e][:, :])

                gg_t, loc = gg_of[e]

                # mm1: hT[:, jt, lo:hi] = relu(w1^T-contract x_slots + b1)
                for jt in range(JT):
                    for a, bnd, p in mm1_chunks(lo, hi):
                        ln = bnd - a
                        pp0 = pieces[p][0]
                        psh = psh_pool.tile([P, 512], F32, tag="psh")
                        for dt_i in range(DT):
                            nc.tensor.matmul(
                                psh[:, 0:ln],
                                lhsT=w1_sb[:, dt_i, jt * P : (jt + 1) * P],
                                rhs=xg_tiles[p][:, dt_i, a - pp0 : bnd - pp0],
                                start=(dt_i == 0),
                                stop=(dt_i == DT - 1),
                            )
                        nc.scalar.activation(
                            hT[:, jt, a:bnd],
                            psh[:, 0:ln],
                            AF.Relu,
                            bias=b1_sb[:, jt : jt + 1],
                        )

                # mm2: y = relu(gate * (hT^T w2 + b2)) ; token(slot)-major psum.
                # b2 pre-loaded into PSUM by DVE; matmuls accumulate on top.
                y_e = ypool.tile([P, NCH, D], F16, tag="y")
                tail = C[e] - P * (nch[e] - 1)
                if tail < P:  # ragged last chunk: init rows the ACT won't write
                    nc.vector.memset(y_e[:, nch[e] - 1, :], 0.0)
                if nch[e] < NCH:
                    nc.vector.memset(y_e[:, nch[e] : NCH, :], 0.0)
                for i in range(nch[e]):
                    la = P * i
                    lb = min(la + P, C[e])
                    ln = lb - la
                    for dc in range(2):
                        psy = psy_pool.tile([P, 512], F32, tag="psy")
                        for jt in range(JT):
                            nc.tensor.matmul(
                                psy[0:ln, :],
                                lhsT=hT[:, jt, lo + la : lo + lb],
                                rhs=w2_sb[:, jt, dc * 512 : (dc + 1) * 512],
                                start=(jt == 0),
                                stop=(jt == JT - 1),
                            )
                        nc.vector.tensor_add(
                            psy[0:ln, :],
                            psy[0:ln, :],
                            b2_sb[0:ln, dc * 512 : (dc + 1) * 512],
                        )
                        nc.scalar.activation(
                            y_e[0:ln, i, dc * 512 : (dc + 1) * 512],
                            psy[0:ln, :],
                            AF.Relu,
                            scale=gg_t[0:ln, loc + i, e : e + 1],
                        )

                nc.gpsimd.dma_scatter_add(
                    out[:, :],
                    y_e[:, 0 : nch[e], :],
                    sidx_sb[:, lo // 16 : hi // 16],
                    C[e],
                    C[e],
                    D,
                )
    nc.compile()
    return nc


_NC_CACHE = {}


def _get_nc(C):
    key = tuple(C)
    if key not in _NC_CACHE:
        _NC_CACHE[key] = build_nc(key)
    return _NC_CACHE[key]


def _topk_idx(x, route_w):
    """Top-4 expert ids per token, matching the reference's jax top_k."""
    try:
        import jax

        cpu = jax.devices("cpu")[0]
        with jax.default_device(cpu):
            f = jax.jit(
                lambda x, r: jax.lax.top_k(x @ r.T, K)[1], backend="cpu"
            )
            return np.asarray(f(x, route_w))
    except Exception:
        l = x.astype(np.float32) @ route_w.astype(np.float32).T
        return np.argsort(-l, axis=1, kind="stable")[:, :K].astype(np.int32)


def _schedule(x, route_w):
    """Build per-core dispatch schedule. Returns (C, per-core arrays)."""
    idx = _topk_idx(np.asarray(x, np.float32), np.asarray(route_w, np.float32))
    sel = np.zeros((NCORES, BL, E), dtype=bool)
    rows = np.arange(BL)
    for c in range(NCORES):
        sel[c, rows[:, None].repeat(K, 1), idx[c * BL : (c + 1) * BL]] = True
    counts = sel.sum(axis=1)  # (NCORES, E)
    C = ((counts.max(axis=0) + 15) // 16 * 16).astype(int)
    C = np.maximum(C, 16)
    C = tuple(int(v) for v in C)
    pieces, piece_of, offs, S, nch, groups, G = _layout(C)

    # idx j -> [j%16, j//16], replicated across the 8 gpsimd cores
    def wrap(a):
        return np.ascontiguousarray(np.tile(a.reshape(-1, 16).T, (8, 1)))

    goffs = np.concatenate([[0], np.cumsum([P * n for n in nch])]).astype(int)
    per_core = []
    for c in range(NCORES):
        g = np.zeros(S, dtype=np.int16)
        s = np.full(S, DUMMY, dtype=np.int16)
        gg = np.zeros(G, dtype=np.int16)
        for e in range(E):
            toks = np.nonzero(sel[c, :, e])[0]
            n = len(toks)
            g[offs[e] : offs[e] + n] = toks
            s[offs[e] : offs[e] + n] = toks
            gg[goffs[e] : goffs[e] + n] = toks
        per_core.append((wrap(g), wrap(s), wrap(gg)))
    return C, per_core


def _prep_in_maps(x, route_w, w1, b1, w2, b2, C, per_core):
    x = np.asarray(x, dtype=np.float32)
    rw = np.asarray(route_w, dtype=np.float32)
    r_hi = rw.T.astype(np.float16)
    r_lo = (rw.T - r_hi.astype(np.float32)).astype(np.float16)
    r_cat = np.ascontiguousarray(np.concatenate([r_hi, r_lo], axis=1))
    w1t = np.ascontiguousarray(
        np.asarray(w1, np.float32).transpose(0, 2, 1).astype(np.float16)
    )
    w2t = np.ascontiguousarray(
        np.asarray(w2, np.float32).transpose(0, 2, 1).astype(np.float16)
    )
    b1 = np.ascontiguousarray(np.asarray(b1, np.float32))
    b2 = np.ascontiguousarray(
        np.repeat(np.asarray(b2, np.float32).astype(np.float16)[:, None, :], P, 1)
    )
    in_maps = []
    for c in range(NCORES):
        xc = x[c * BL : (c + 1) * BL]
        gw, sw, ggw = per_core[c]
        in_maps.append(
            {
                "x_row": np.ascontiguousarray(xc.astype(np.float16)),
                "x_t": np.ascontiguousarray(xc.T.astype(np.float16)),
                "r_cat": r_cat,
                "w1t": w1t,
                "w2t": w2t,
                "b1": b1,
                "b2": b2,
                "gidx": gw,
                "sidx": sw,
                "ggidx": ggw,
            }
        )
    return in_maps


def run(x, route_w, w1, b1, w2, b2, trace=False, **trace_kw):
    C, per_core = _schedule(x, route_w)
    nc = _get_nc(C)
    in_maps = _prep_in_maps(x, route_w, w1, b1, w2, b2, C, per_core)
    res = run_bass_kernel_spmd(
        nc, in_maps, list(range(NCORES)), trace=trace, **trace_kw
    )
    out = np.concatenate(
        [r["out"][:BL].astype(np.float32) for r in res.results], axis=0
    )
    return out, res


def kernel(x, route_w, w1, b1, w2, b2):
    out, _ = run(x, route_w, w1, b1, w2, b2, trace=False)
    return out
